# revision 5
# baseline (speedup 1.0000x reference)
"""ArcFace loss on 8 Trainium2 NeuronCores (vocab/tensor-parallel over C).

Math (reference):
    logits = features @ w                       # [B, C]
    modulus[b,c] = |features[b]| * |w[:,c]|
    cos = logits / modulus / 1.01
    margin_logits = modulus * cos(arccos(cos) + ANGLE)
    top = exp(margin_logits[b, t_b])
    down = sum_c exp(logits[b,c]) - exp(logits[b,t_b]) + top
    loss = -mean_b log(top / down)

Only the row-sum of exp(logits) touches all of [B, C]; the margin math is
needed only at the target column of each row.  cos(arccos(x)+m) is expanded
as x*cos(m) - sin(m)*sqrt(1-x^2), so:
    log top = margin_b = cos(m)/1.01 * gl_b - sin(m)*sqrt(fm2_b*gm2_b - (gl_b/1.01)^2)
with gl_b = logits[b, t_b], fm2_b = |f_b|^2, gm2_b = |w_col(t_b)|^2.

Sharding: w is split over the category axis, 12500 columns per core.  Each
core computes per-row partial exp-sums (fused into the ScalarE exp pass via
accum_out), gathers its locally-owned target columns with an indirect DMA
(masked to zero for rows owned by other cores), and a single 6 KB AllReduce
combines [rowsum, gl, gm2] across cores.  The scalar loss epilogue then runs
replicated on every core.
"""

import numpy as np

try:
    import concourse.bass as bass
except ImportError:
    import sys

    sys.path.insert(0, "/opt/trn_rl_repo")
    import concourse.bass as bass

import concourse.mybir as mybir
import concourse.tile as tile
from concourse import bacc
from concourse.bass import IndirectOffsetOnAxis
from concourse.bass_utils import run_bass_kernel_spmd
from concourse.masks import make_identity

B, F, C = 512, 128, 100000
NCORES = 8
CS = C // NCORES  # 12500 columns per core
BT = B // 128  # 4 row tiles
ANGLE = 0.5
COS_M = float(np.cos(ANGLE))
SIN_M = float(np.sin(ANGLE))
INV_S = 1.0 / 1.01

NT = 512  # matmul free-dim tile (one PSUM bank of fp32)
N_FULL = CS // NT  # 24 full tiles
COL_TILES = [NT] * N_FULL + ([CS - N_FULL * NT] if CS % NT else [])
GROUPS = [COL_TILES[i : i + 4] for i in range(0, len(COL_TILES), 4)]
NG = len(GROUPS)  # 7 (6 x 2048 cols + 1 x 212 cols)

f32 = mybir.dt.float32
i32 = mybir.dt.int32
AX = mybir.AxisListType
ALU = mybir.AluOpType
ACTF = mybir.ActivationFunctionType


def _body(tc, feat, w, wt_dram, tidx, tmask, out):
    nc = tc.nc
    with (
        tc.tile_pool(name="persist", bufs=1) as sb,
        tc.tile_pool(name="scratch", bufs=3) as scratch,
        tc.tile_pool(name="psum", bufs=2, space="PSUM") as pp,
        tc.tile_pool(name="dram", bufs=1, space="DRAM") as dp,
    ):
        # ---- persistent SBUF tiles ----
        f_sb = sb.tile([128, B], f32, tag="f_sb")  # features, b-major tiles
        fT = sb.tile([F, B], f32, tag="fT")  # features^T (matmul lhsT)
        ident = sb.tile([128, 128], f32, tag="ident")
        ones = sb.tile([128, 1], f32, tag="ones")
        tidx_sb = sb.tile([128, BT], i32, tag="tidx_sb")
        tmask_sb = sb.tile([128, BT], f32, tag="tmask_sb")
        acc = sb.tile([128, BT * NG], f32, tag="acc")  # exp row-sum partials
        glog = sb.tile([128, BT], f32, tag="glog")  # gathered logit (masked)
        gm2 = sb.tile([128, BT], f32, tag="gm2")  # gathered |w_col|^2 (masked)
        fm2 = sb.tile([128, BT], f32, tag="fm2")  # |f_row|^2 (local)
        pack = sb.tile([128, 3 * BT], f32, tag="pack")
        red = sb.tile([128, 3 * BT], f32, tag="red")
        epi = sb.tile([128, 10 * BT], f32, tag="epi")  # epilogue scratch
        res_sb = sb.tile([1, 1], f32, tag="res_sb")

        # ---- stream w into SBUF, one chunk per column group ----
        wchunks = []
        c0 = 0
        for g, group in enumerate(GROUPS):
            gw = sum(group)
            wt = sb.tile([128, gw], f32, tag=f"wchunk{g}")
            nc.sync.dma_start(wt[:], w[:, c0 : c0 + gw])
            wchunks.append(wt)
            c0 += gw

        # ---- small loads + constants ----
        nc.sync.dma_start(tidx_sb[:], tidx[:, :])
        nc.sync.dma_start(tmask_sb[:], tmask[:, :])
        # f_sb[p, t*128 + k] = feat[t*128 + p, k]
        nc.sync.dma_start(f_sb[:], feat.ap().rearrange("(t p) k -> p t k", t=BT))
        make_identity(nc, ident[:])
        nc.gpsimd.memset(ones[:], 1.0)

        # ---- per row-tile: transpose features; gather target column; dots ----
        wT_view = wt_dram.ap()  # [CS, F]; row c is w[:, c], contiguous
        for bt in range(BT):
            f_bt = f_sb[:, bt * F : (bt + 1) * F]
            ps = pp.tile([128, 2048], f32, tag="psum", name=f"ps_t{bt}")
            nc.tensor.transpose(out=ps[:, 0:128], in_=f_bt, identity=ident[:])
            nc.vector.tensor_copy(out=fT[:, bt * 128 : (bt + 1) * 128], in_=ps[:, 0:128])

            junk0 = scratch.tile([128, F], f32, tag="dots")
            nc.vector.scalar_tensor_tensor(
                out=junk0[:], in0=f_bt, scalar=1.0, in1=f_bt,
                op0=ALU.mult, op1=ALU.mult, accum_out=fm2[:, bt : bt + 1],
            )

            wg = scratch.tile([128, F], f32, tag="wg")
            nc.gpsimd.indirect_dma_start(
                out=wg[:], out_offset=None,
                in_=wT_view,
                in_offset=IndirectOffsetOnAxis(ap=tidx_sb[:, bt : bt + 1], axis=0),
            )
            junk1 = scratch.tile([128, F], f32, tag="dots")
            nc.vector.scalar_tensor_tensor(
                out=junk1[:], in0=wg[:], scalar=tmask_sb[:, bt : bt + 1], in1=f_bt,
                op0=ALU.mult, op1=ALU.mult, accum_out=glog[:, bt : bt + 1],
            )
            junk2 = scratch.tile([128, F], f32, tag="dots")
            nc.vector.scalar_tensor_tensor(
                out=junk2[:], in0=wg[:], scalar=tmask_sb[:, bt : bt + 1], in1=wg[:],
                op0=ALU.mult, op1=ALU.mult, accum_out=gm2[:, bt : bt + 1],
            )

        # ---- main loop: matmul -> exp (+row-sum accumulate) ----
        for g, group in enumerate(GROUPS):
            wt = wchunks[g]
            gw = sum(group)
            for bt in range(BT):
                ps = pp.tile([128, 2048], f32, tag="psum", name=f"ps_{g}_{bt}")
                off = 0
                for n in group:
                    nc.tensor.matmul(
                        out=ps[:, off : off + n],
                        lhsT=fT[:, bt * 128 : (bt + 1) * 128],
                        rhs=wt[:, off : off + n],
                        start=True, stop=True,
                    )
                    off += n
                ex = scratch.tile([128, 2048], f32, tag="exp", name=f"ex_{g}_{bt}")
                col = bt * NG + g
                nc.scalar.activation(
                    out=ex[:, :gw], in_=ps[:, :gw], func=ACTF.Exp,
                    accum_out=acc[:, col : col + 1],
                )

        # ---- pack partials and AllReduce across the 8 cores ----
        for bt in range(BT):
            nc.vector.reduce_sum(
                out=pack[:, bt : bt + 1],
                in_=acc[:, bt * NG : (bt + 1) * NG],
                axis=AX.X,
            )
        nc.vector.tensor_copy(out=pack[:, BT : 2 * BT], in_=glog[:])
        nc.vector.tensor_copy(out=pack[:, 2 * BT : 3 * BT], in_=gm2[:])

        cc_in = dp.tile([128, 3 * BT], f32, tag="cc_in")
        cc_out = dp.tile([128, 3 * BT], f32, tag="cc_out", addr_space="Shared")
        nc.sync.dma_start(cc_in[:], pack[:])
        nc.gpsimd.collective_compute(
            "AllReduce", ALU.add,
            replica_groups=[list(range(NCORES))],
            ins=[cc_in[:].opt()], outs=[cc_out[:].opt()],
        )
        nc.sync.dma_start(red[:], cc_out[:])

        # ---- epilogue (identical on every core), [128, BT] layout ----
        rs = red[:, 0:BT]
        gl = red[:, BT : 2 * BT]
        g2 = red[:, 2 * BT : 3 * BT]

        def lane(i):
            return epi[:, i * BT : (i + 1) * BT]

        a, asq, m2, t2, root, amc, margin, egl, down, ld = (lane(i) for i in range(10))

        nc.vector.tensor_scalar_mul(a, gl, INV_S)  # a = gl/1.01
        nc.vector.tensor_tensor(out=asq, in0=a, in1=a, op=ALU.mult)
        nc.vector.tensor_tensor(out=m2, in0=fm2[:], in1=g2, op=ALU.mult)
        nc.vector.tensor_tensor(out=t2, in0=m2, in1=asq, op=ALU.subtract)
        # root = sqrt(t2) = exp(0.5*ln(t2)); keeps ScalarE in the ln/exp table set
        nc.scalar.activation(out=root, in_=t2, func=ACTF.Ln)
        nc.scalar.activation(out=root, in_=root, func=ACTF.Exp, scale=0.5)
        nc.vector.tensor_scalar_mul(amc, a, COS_M)
        nc.vector.scalar_tensor_tensor(
            out=margin, in0=root, scalar=-SIN_M, in1=amc, op0=ALU.mult, op1=ALU.add
        )
        nc.scalar.activation(out=egl, in_=gl, func=ACTF.Exp)
        nc.vector.tensor_tensor(out=down, in0=rs, in1=egl, op=ALU.subtract)
        nc.scalar.activation(out=egl, in_=margin, func=ACTF.Exp)  # reuse: exp(margin)
        nc.vector.tensor_tensor(out=down, in0=down, in1=egl, op=ALU.add)
        nc.scalar.activation(out=ld, in_=down, func=ACTF.Ln)
        nc.vector.tensor_tensor(out=margin, in0=margin, in1=ld, op=ALU.subtract)

        colsum = epi[:, 0:1]  # reuse lane 0 col 0
        nc.vector.reduce_sum(out=colsum, in_=margin, axis=AX.X)
        ps = pp.tile([128, 2048], f32, tag="psum", name="ps_final")
        nc.tensor.matmul(
            out=ps[:1, 0:1], lhsT=colsum, rhs=ones[:], start=True, stop=True
        )
        nc.scalar.mul(res_sb[:], ps[:1, 0:1], -1.0 / B)
        nc.sync.dma_start(out[:, :], res_sb[:])


_CACHED_NC = None


def build():
    global _CACHED_NC
    if _CACHED_NC is not None:
        return _CACHED_NC
    nc = bacc.Bacc(
        "TRN2", target_bir_lowering=False, debug=False, num_devices=NCORES
    )
    feat = nc.dram_tensor("features", [B, F], f32, kind="ExternalInput")
    w = nc.dram_tensor("w", [F, CS], f32, kind="ExternalInput")
    wt = nc.dram_tensor("wt", [CS, F], f32, kind="ExternalInput")
    tidx = nc.dram_tensor("tidx", [128, BT], i32, kind="ExternalInput")
    tmask = nc.dram_tensor("tmask", [128, BT], f32, kind="ExternalInput")
    out = nc.dram_tensor("out", [1, 1], f32, kind="ExternalOutput")
    with tile.TileContext(nc) as tc:
        _body(tc, feat, w, wt, tidx, tmask, out)
    nc.compile()
    _CACHED_NC = nc
    return nc


def make_in_maps(features, w, target):
    features = np.ascontiguousarray(np.asarray(features, dtype=np.float32))
    w = np.asarray(w, dtype=np.float32)
    tgt = np.asarray(target).astype(np.int64).ravel()
    in_maps = []
    for m in range(NCORES):
        base = m * CS
        local = (tgt >= base) & (tgt < base + CS)
        tid = np.where(local, tgt - base, 0).astype(np.int32)
        msk = local.astype(np.float32)
        wshard = np.ascontiguousarray(w[:, base : base + CS])
        in_maps.append(
            {
                "features": features,
                "w": wshard,
                "wt": np.ascontiguousarray(wshard.T),
                # [128, BT] b-major: [p, t] -> row t*128+p
                "tidx": np.ascontiguousarray(tid.reshape(BT, 128).T),
                "tmask": np.ascontiguousarray(msk.reshape(BT, 128).T),
            }
        )
    return in_maps


def run(features, w, target, **kwargs):
    nc = build()
    in_maps = make_in_maps(features, w, target)
    return run_bass_kernel_spmd(nc, in_maps, core_ids=list(range(NCORES)), **kwargs)


def kernel(features, w, target):
    res = run(features, w, target)
    val = np.float32(res.results[0]["out"][0, 0])
    return np.array(val, dtype=np.float32)


# revision 8
# speedup vs baseline: 1.3138x; 1.3138x over previous
"""ArcFace loss on 8 Trainium2 NeuronCores (vocab/tensor-parallel over C).

Math (reference):
    logits = features @ w                       # [B, C]
    modulus[b,c] = |features[b]| * |w[:,c]|
    cos = logits / modulus / 1.01
    margin_logits = modulus * cos(arccos(cos) + ANGLE)
    top = exp(margin_logits[b, t_b])
    down = sum_c exp(logits[b,c]) - exp(logits[b,t_b]) + top
    loss = -mean_b log(top / down)

Only the row-sum of exp(logits) touches all of [B, C]; the margin math is
needed only at the target column of each row.  cos(arccos(x)+m) is expanded
as x*cos(m) - sin(m)*sqrt(1-x^2), giving
    log top = margin_b = cos(m)/1.01 * gl_b - sin(m)*sqrt(fm2_b*gm2_b - (gl_b/1.01)^2)
with gl_b = logits[b, t_b], fm2_b = |f_b|^2, gm2_b = |w_col(t_b)|^2.

Sharding: w is split over the category axis, 12500 columns per core.  Each
core:
  - streams its w shard through TensorE (bf16, cast during DMA) against
    features^T, ScalarE exponentiates straight out of PSUM with the row-sum
    fused via accum_out (the [B,C] intermediate never exists in HBM);
  - gathers its locally-owned target columns with an indirect DMA and
    computes masked margin / exp(gl) / exp(margin) per row — rows owned by
    other cores contribute exact zeros;
  - one 8 KB AllGather exchanges [rowsum, margin, egl, etop] partials;
    every core then sums the 8 shards and finishes the scalar loss
    replicated (down = rowsum - egl + etop; loss = -mean(margin - ln(down))).
"""

import numpy as np

try:
    import concourse.bass as bass
except ImportError:
    import sys

    sys.path.insert(0, "/opt/trn_rl_repo")
    import concourse.bass as bass

import concourse.mybir as mybir
import concourse.tile as tile
from concourse import bacc
from concourse.bass import IndirectOffsetOnAxis
from concourse.bass_utils import run_bass_kernel_spmd
from concourse.masks import make_identity

B, F, C = 512, 128, 100000
NCORES = 8
CS = C // NCORES  # 12500 columns per core
BT = B // 128  # 4 row tiles
ANGLE = 0.5
COS_M = float(np.cos(ANGLE))
SIN_M = float(np.sin(ANGLE))
INV_S = 1.0 / 1.01

NT = 512  # matmul free-dim tile (one PSUM bank of fp32)
N_FULL = CS // NT  # 24 full tiles
COL_TILES = [NT] * N_FULL + ([CS - N_FULL * NT] if CS % NT else [])
GROUPS = [COL_TILES[i : i + 4] for i in range(0, len(COL_TILES), 4)]
NG = len(GROUPS)  # 7 (6 x 2048 cols + 1 x 212 cols)

f32 = mybir.dt.float32
bf16 = mybir.dt.bfloat16
i32 = mybir.dt.int32
AX = mybir.AxisListType
ALU = mybir.AluOpType
ACTF = mybir.ActivationFunctionType

NPACK = 4 * BT  # rowsum, margin, egl, etop


def _body(tc, feat, w, wt_dram, tidx, tmask, out):
    nc = tc.nc
    with (
        tc.tile_pool(name="persist", bufs=1) as sb,
        tc.tile_pool(name="scratch", bufs=3) as scratch,
        tc.tile_pool(name="psum", bufs=2, space="PSUM") as pp,
        tc.tile_pool(name="dram", bufs=1, space="DRAM") as dp,
    ):
        # ---- persistent SBUF tiles ----
        f_sb = sb.tile([128, B], f32, tag="f_sb")  # features, b-major tiles
        fT = sb.tile([F, B], bf16, tag="fT")  # features^T (matmul lhsT)
        ident = sb.tile([128, 128], f32, tag="ident")
        ones = sb.tile([128, 1], f32, tag="ones")
        tidx_sb = sb.tile([128, BT], i32, tag="tidx_sb")
        tmask_sb = sb.tile([128, BT], f32, tag="tmask_sb")
        acc = sb.tile([128, BT * NG], f32, tag="acc")  # exp row-sum partials
        glog = sb.tile([128, BT], f32, tag="glog")  # gathered logit (masked)
        gm2 = sb.tile([128, BT], f32, tag="gm2")  # gathered |w_col|^2 (masked)
        fm2 = sb.tile([128, BT], f32, tag="fm2")  # |f_row|^2 (local)
        pack = sb.tile([128, NPACK], f32, tag="pack")
        red8 = sb.tile([128, NCORES * NPACK], f32, tag="red8")
        epi = sb.tile([128, 12 * BT], f32, tag="epi")  # epilogue scratch
        res_sb = sb.tile([1, 1], f32, tag="res_sb")

        # ---- stream w into SBUF (cast f32 -> bf16 in the DMA), one chunk
        # per column group ----
        wchunks = []
        c0 = 0
        for g, group in enumerate(GROUPS):
            gw = sum(group)
            wtile = sb.tile([128, gw], bf16, tag=f"wchunk{g}")
            nc.gpsimd.dma_start(wtile[:], w[:, c0 : c0 + gw])
            wchunks.append(wtile)
            c0 += gw

        # ---- small loads + constants ----
        nc.sync.dma_start(tidx_sb[:], tidx[:, :])
        nc.sync.dma_start(tmask_sb[:], tmask[:, :])
        # f_sb[p, t*128 + k] = feat[t*128 + p, k]
        nc.sync.dma_start(f_sb[:], feat.ap().rearrange("(t p) k -> p t k", t=BT))
        make_identity(nc, ident[:])
        nc.gpsimd.memset(ones[:], 1.0)

        # ---- per row-tile: transpose features; gather target column; dots ----
        wT_view = wt_dram.ap()  # [CS, F]; row c is w[:, c], contiguous
        for bt in range(BT):
            f_bt = f_sb[:, bt * F : (bt + 1) * F]
            ps = pp.tile([128, 2048], f32, tag="psum", name=f"ps_t{bt}")
            nc.tensor.transpose(out=ps[:, 0:128], in_=f_bt, identity=ident[:])
            # cast to bf16 on the way out of PSUM
            nc.vector.tensor_copy(out=fT[:, bt * 128 : (bt + 1) * 128], in_=ps[:, 0:128])

            junk0 = scratch.tile([128, F], f32, tag="dots")
            nc.vector.scalar_tensor_tensor(
                out=junk0[:], in0=f_bt, scalar=1.0, in1=f_bt,
                op0=ALU.mult, op1=ALU.mult, accum_out=fm2[:, bt : bt + 1],
            )

            wg = scratch.tile([128, F], f32, tag="wg")
            nc.gpsimd.indirect_dma_start(
                out=wg[:], out_offset=None,
                in_=wT_view,
                in_offset=IndirectOffsetOnAxis(ap=tidx_sb[:, bt : bt + 1], axis=0),
            )
            junk1 = scratch.tile([128, F], f32, tag="dots")
            nc.vector.scalar_tensor_tensor(
                out=junk1[:], in0=wg[:], scalar=tmask_sb[:, bt : bt + 1], in1=f_bt,
                op0=ALU.mult, op1=ALU.mult, accum_out=glog[:, bt : bt + 1],
            )
            junk2 = scratch.tile([128, F], f32, tag="dots")
            nc.vector.scalar_tensor_tensor(
                out=junk2[:], in0=wg[:], scalar=tmask_sb[:, bt : bt + 1], in1=wg[:],
                op0=ALU.mult, op1=ALU.mult, accum_out=gm2[:, bt : bt + 1],
            )

        # ---- pre-collective masked epilogue (hidden under the main loop) ----
        # For rows owned elsewhere: glog = gm2 = 0, mask = 0; every quantity
        # below is finite and the masked outputs are exact zeros.
        def lane(i):
            return epi[:, i * BT : (i + 1) * BT]

        a, t2, root, amc, margin, nmask, tmp = (lane(i) for i in range(7))
        nc.vector.tensor_scalar_mul(a, glog[:], INV_S)  # a = gl/1.01
        nc.vector.tensor_tensor(out=t2, in0=fm2[:], in1=gm2[:], op=ALU.mult)
        nc.vector.tensor_tensor(out=tmp, in0=a, in1=a, op=ALU.mult)
        nc.vector.tensor_tensor(out=t2, in0=t2, in1=tmp, op=ALU.subtract)
        # nmask = 1 - mask;  t2 += nmask so unowned rows stay > 0
        nc.vector.tensor_scalar(
            out=nmask, in0=tmask_sb[:], scalar1=-1.0, scalar2=1.0,
            op0=ALU.mult, op1=ALU.add,
        )
        nc.vector.tensor_tensor(out=t2, in0=t2, in1=nmask, op=ALU.add)
        # root = sqrt(t2) = exp(0.5*ln(t2))
        nc.scalar.activation(out=root, in_=t2, func=ACTF.Ln)
        nc.scalar.activation(out=root, in_=root, func=ACTF.Exp, scale=0.5)
        nc.vector.tensor_scalar_mul(amc, a, COS_M)
        nc.vector.scalar_tensor_tensor(
            out=margin, in0=root, scalar=-SIN_M, in1=amc, op0=ALU.mult, op1=ALU.add
        )
        # masked AG contributions: margin_m, egl_m = mask*exp(gl), etop_m = mask*exp(margin)
        nc.scalar.activation(out=tmp, in_=glog[:], func=ACTF.Exp)
        nc.vector.tensor_tensor(
            out=pack[:, 2 * BT : 3 * BT], in0=tmp, in1=tmask_sb[:], op=ALU.mult
        )
        nc.scalar.activation(out=tmp, in_=margin, func=ACTF.Exp)
        nc.vector.tensor_tensor(
            out=pack[:, 3 * BT : 4 * BT], in0=tmp, in1=tmask_sb[:], op=ALU.mult
        )
        nc.vector.tensor_tensor(
            out=pack[:, BT : 2 * BT], in0=margin, in1=tmask_sb[:], op=ALU.mult
        )

        # ---- main loop: matmul -> exp (+row-sum accumulate) ----
        for g, group in enumerate(GROUPS):
            wtile = wchunks[g]
            gw = sum(group)
            for bt in range(BT):
                ps = pp.tile([128, 2048], f32, tag="psum", name=f"ps_{g}_{bt}")
                off = 0
                for n in group:
                    nc.tensor.matmul(
                        out=ps[:, off : off + n],
                        lhsT=fT[:, bt * 128 : (bt + 1) * 128],
                        rhs=wtile[:, off : off + n],
                        start=True, stop=True,
                    )
                    off += n
                ex = scratch.tile([128, 2048], f32, tag="exp", name=f"ex_{g}_{bt}")
                col = bt * NG + g
                nc.scalar.activation(
                    out=ex[:, :gw], in_=ps[:, :gw], func=ACTF.Exp,
                    accum_out=acc[:, col : col + 1],
                )

        # ---- pack row-sum partials and AllGather across the 8 cores ----
        for bt in range(BT):
            nc.vector.reduce_sum(
                out=pack[:, bt : bt + 1],
                in_=acc[:, bt * NG : (bt + 1) * NG],
                axis=AX.X,
            )

        cc_in = dp.tile([128, NPACK], f32, tag="cc_in")
        cc_out = dp.tile([NCORES, 128, NPACK], f32, tag="cc_out", addr_space="Shared")
        nc.sync.dma_start(cc_in[:], pack[:])
        nc.gpsimd.collective_compute(
            "AllGather", ALU.bypass,
            replica_groups=[list(range(NCORES))],
            ins=[cc_in[:].opt()], outs=[cc_out[:].opt()],
        )
        # red8[p, m*NPACK + j] = cc_out[m, p, j]
        nc.sync.dma_start(red8[:], cc_out[:].rearrange("m p j -> p m j"))

        # ---- post-collective tail (replicated on every core) ----
        red = epi[:, 7 * BT : 7 * BT + NPACK]  # summed over cores
        nc.vector.reduce_sum(
            out=red,
            in_=red8[:].rearrange("p (m j) -> p j m", m=NCORES),
            axis=AX.X,
        )
        rs = red[:, 0:BT]
        marg = red[:, BT : 2 * BT]
        egl = red[:, 2 * BT : 3 * BT]
        etop = red[:, 3 * BT : 4 * BT]
        down = lane(0)  # reuse
        nc.vector.tensor_tensor(out=down, in0=rs, in1=egl, op=ALU.subtract)
        nc.vector.tensor_tensor(out=down, in0=down, in1=etop, op=ALU.add)
        ld = lane(1)
        nc.scalar.activation(out=ld, in_=down, func=ACTF.Ln)
        val = lane(2)
        nc.vector.tensor_tensor(out=val, in0=marg, in1=ld, op=ALU.subtract)
        colsum = lane(3)[:, 0:1]
        nc.vector.reduce_sum(out=colsum, in_=val, axis=AX.X)
        ps = pp.tile([128, 2048], f32, tag="psum", name="ps_final")
        nc.tensor.matmul(
            out=ps[:1, 0:1], lhsT=colsum, rhs=ones[:], start=True, stop=True
        )
        nc.scalar.mul(res_sb[:], ps[:1, 0:1], -1.0 / B)
        nc.sync.dma_start(out[:, :], res_sb[:])


_CACHED_NC = None


def build():
    global _CACHED_NC
    if _CACHED_NC is not None:
        return _CACHED_NC
    nc = bacc.Bacc(
        "TRN2", target_bir_lowering=False, debug=False, num_devices=NCORES
    )
    feat = nc.dram_tensor("features", [B, F], f32, kind="ExternalInput")
    w = nc.dram_tensor("w", [F, CS], f32, kind="ExternalInput")
    wt = nc.dram_tensor("wt", [CS, F], f32, kind="ExternalInput")
    tidx = nc.dram_tensor("tidx", [128, BT], i32, kind="ExternalInput")
    tmask = nc.dram_tensor("tmask", [128, BT], f32, kind="ExternalInput")
    out = nc.dram_tensor("out", [1, 1], f32, kind="ExternalOutput")
    with tile.TileContext(nc) as tc:
        _body(tc, feat, w, wt, tidx, tmask, out)
    nc.compile()
    _CACHED_NC = nc
    return nc


def make_in_maps(features, w, target):
    features = np.ascontiguousarray(np.asarray(features, dtype=np.float32))
    w = np.asarray(w, dtype=np.float32)
    tgt = np.asarray(target).astype(np.int64).ravel()
    in_maps = []
    for m in range(NCORES):
        base = m * CS
        local = (tgt >= base) & (tgt < base + CS)
        tid = np.where(local, tgt - base, 0).astype(np.int32)
        msk = local.astype(np.float32)
        wshard = np.ascontiguousarray(w[:, base : base + CS])
        in_maps.append(
            {
                "features": features,
                "w": wshard,
                "wt": np.ascontiguousarray(wshard.T),
                # [128, BT] b-major: [p, t] -> row t*128+p
                "tidx": np.ascontiguousarray(tid.reshape(BT, 128).T),
                "tmask": np.ascontiguousarray(msk.reshape(BT, 128).T),
            }
        )
    return in_maps


def run(features, w, target, **kwargs):
    nc = build()
    in_maps = make_in_maps(features, w, target)
    return run_bass_kernel_spmd(nc, in_maps, core_ids=list(range(NCORES)), **kwargs)


def kernel(features, w, target):
    res = run(features, w, target)
    val = np.float32(res.results[0]["out"][0, 0])
    return np.array(val, dtype=np.float32)


# revision 10
# speedup vs baseline: 1.3711x; 1.0436x over previous
"""ArcFace loss on 8 Trainium2 NeuronCores (vocab/tensor-parallel over C).

Math (reference):
    logits = features @ w                       # [B, C]
    modulus[b,c] = |features[b]| * |w[:,c]|
    cos = logits / modulus / 1.01
    margin_logits = modulus * cos(arccos(cos) + ANGLE)
    top = exp(margin_logits[b, t_b])
    down = sum_c exp(logits[b,c]) - exp(logits[b,t_b]) + top
    loss = -mean_b log(top / down)

Only the row-sum of exp(logits) touches all of [B, C]; the margin math is
needed only at the target column of each row.  cos(arccos(x)+m) is expanded
as x*cos(m) - sin(m)*sqrt(1-x^2), giving
    log top = margin_b = cos(m)/1.01 * gl_b - sin(m)*sqrt(fm2_b*gm2_b - (gl_b/1.01)^2)
with gl_b = logits[b, t_b], fm2_b = |f_b|^2, gm2_b = |w_col(t_b)|^2.

Sharding: w is split over the category axis, 12500 columns per core.  Each
core:
  - streams its w shard through TensorE (bf16, cast during DMA) against
    features^T, ScalarE exponentiates straight out of PSUM with the row-sum
    fused via accum_out (the [B,C] intermediate never exists in HBM);
  - gathers its locally-owned target columns with an indirect DMA and
    computes masked margin / exp(gl) / exp(margin) per row — rows owned by
    other cores contribute exact zeros;
  - one 8 KB AllGather exchanges [rowsum, margin, egl, etop] partials;
    every core then sums the 8 shards and finishes the scalar loss
    replicated (down = rowsum - egl + etop; loss = -mean(margin - ln(down))).
"""

import numpy as np

try:
    import concourse.bass as bass
except ImportError:
    import sys

    sys.path.insert(0, "/opt/trn_rl_repo")
    import concourse.bass as bass

import concourse.mybir as mybir
import concourse.tile as tile
from concourse import bacc
from concourse.bass import IndirectOffsetOnAxis
from concourse.bass_utils import run_bass_kernel_spmd
from concourse.masks import make_identity

B, F, C = 512, 128, 100000
NCORES = 8
CS = C // NCORES  # 12500 columns per core
BT = B // 128  # 4 row tiles
ANGLE = 0.5
COS_M = float(np.cos(ANGLE))
SIN_M = float(np.sin(ANGLE))
INV_S = 1.0 / 1.01

NT = 512  # matmul free-dim tile (one PSUM bank of fp32)
N_FULL = CS // NT  # 24 full tiles
COL_TILES = [NT] * N_FULL + ([CS - N_FULL * NT] if CS % NT else [])
GROUPS = [COL_TILES[i : i + 4] for i in range(0, len(COL_TILES), 4)]
NG = len(GROUPS)  # 7 (6 x 2048 cols + 1 x 212 cols)

f32 = mybir.dt.float32
bf16 = mybir.dt.bfloat16
i32 = mybir.dt.int32
AX = mybir.AxisListType
ALU = mybir.AluOpType
ACTF = mybir.ActivationFunctionType

NPACK = 4 * BT  # rowsum, margin, egl, etop


def _body(tc, feat, w, wt_dram, tidx, tmask, out):
    nc = tc.nc
    with (
        tc.tile_pool(name="persist", bufs=1) as sb,
        tc.tile_pool(name="scratch", bufs=3) as scratch,
        tc.tile_pool(name="wstage", bufs=3) as wstage,
        tc.tile_pool(name="psum", bufs=2, space="PSUM") as pp,
        tc.tile_pool(name="dram", bufs=1, space="DRAM") as dp,
    ):
        # ---- persistent SBUF tiles ----
        f_sb = sb.tile([128, B], f32, tag="f_sb")  # features, b-major tiles
        fT = sb.tile([F, B], bf16, tag="fT")  # features^T (matmul lhsT)
        ident = sb.tile([128, 128], f32, tag="ident")
        ones = sb.tile([128, 1], f32, tag="ones")
        tidx_sb = sb.tile([128, BT], i32, tag="tidx_sb")
        tmask_sb = sb.tile([128, BT], f32, tag="tmask_sb")
        acc = sb.tile([128, BT * NG], f32, tag="acc")  # exp row-sum partials
        glog = sb.tile([128, BT], f32, tag="glog")  # gathered logit (masked)
        gm2 = sb.tile([128, BT], f32, tag="gm2")  # gathered |w_col|^2 (masked)
        fm2 = sb.tile([128, BT], f32, tag="fm2")  # |f_row|^2 (local)
        pack = sb.tile([128, NPACK], f32, tag="pack")
        red8 = sb.tile([128, NCORES * NPACK], f32, tag="red8")
        epi = sb.tile([128, 12 * BT], f32, tag="epi")  # epilogue scratch
        res_sb = sb.tile([1, 1], f32, tag="res_sb")

        # ---- stream w into SBUF via HWDGE (f32), cast to bf16 on DVE ----
        # (a casting SWDGE DMA measures ~7x slower than HWDGE + DVE copy)
        wchunks = []
        c0 = 0
        for g, group in enumerate(GROUPS):
            gw = sum(group)
            stage = wstage.tile([128, 2048], f32, tag="wstage", name=f"wstage{g}")
            nc.sync.dma_start(stage[:, :gw], w[:, c0 : c0 + gw])
            wtile = sb.tile([128, gw], bf16, tag=f"wchunk{g}")
            nc.vector.tensor_copy(out=wtile[:], in_=stage[:, :gw])
            wchunks.append(wtile)
            c0 += gw

        # ---- small loads + constants ----
        nc.sync.dma_start(tidx_sb[:], tidx[:, :])
        nc.sync.dma_start(tmask_sb[:], tmask[:, :])
        # f_sb[p, t*128 + k] = feat[t*128 + p, k]
        nc.sync.dma_start(f_sb[:], feat.ap().rearrange("(t p) k -> p t k", t=BT))
        make_identity(nc, ident[:])
        nc.gpsimd.memset(ones[:], 1.0)

        # ---- per row-tile: transpose features; gather target column; dots ----
        wT_view = wt_dram.ap()  # [CS, F]; row c is w[:, c], contiguous
        for bt in range(BT):
            f_bt = f_sb[:, bt * F : (bt + 1) * F]
            ps = pp.tile([128, 2048], f32, tag="psum", name=f"ps_t{bt}")
            nc.tensor.transpose(out=ps[:, 0:128], in_=f_bt, identity=ident[:])
            # cast to bf16 on the way out of PSUM
            nc.vector.tensor_copy(out=fT[:, bt * 128 : (bt + 1) * 128], in_=ps[:, 0:128])

            junk0 = scratch.tile([128, F], f32, tag="dots")
            nc.vector.scalar_tensor_tensor(
                out=junk0[:], in0=f_bt, scalar=1.0, in1=f_bt,
                op0=ALU.mult, op1=ALU.mult, accum_out=fm2[:, bt : bt + 1],
            )

            wg = scratch.tile([128, F], f32, tag="wg")
            nc.gpsimd.indirect_dma_start(
                out=wg[:], out_offset=None,
                in_=wT_view,
                in_offset=IndirectOffsetOnAxis(ap=tidx_sb[:, bt : bt + 1], axis=0),
            )
            junk1 = scratch.tile([128, F], f32, tag="dots")
            nc.vector.scalar_tensor_tensor(
                out=junk1[:], in0=wg[:], scalar=tmask_sb[:, bt : bt + 1], in1=f_bt,
                op0=ALU.mult, op1=ALU.mult, accum_out=glog[:, bt : bt + 1],
            )
            junk2 = scratch.tile([128, F], f32, tag="dots")
            nc.vector.scalar_tensor_tensor(
                out=junk2[:], in0=wg[:], scalar=tmask_sb[:, bt : bt + 1], in1=wg[:],
                op0=ALU.mult, op1=ALU.mult, accum_out=gm2[:, bt : bt + 1],
            )

        # ---- pre-collective masked epilogue (hidden under the main loop) ----
        # For rows owned elsewhere: glog = gm2 = 0, mask = 0; every quantity
        # below is finite and the masked outputs are exact zeros.
        def lane(i):
            return epi[:, i * BT : (i + 1) * BT]

        a, t2, root, amc, margin, nmask, tmp = (lane(i) for i in range(7))
        nc.vector.tensor_scalar_mul(a, glog[:], INV_S)  # a = gl/1.01
        nc.vector.tensor_tensor(out=t2, in0=fm2[:], in1=gm2[:], op=ALU.mult)
        nc.vector.tensor_tensor(out=tmp, in0=a, in1=a, op=ALU.mult)
        nc.vector.tensor_tensor(out=t2, in0=t2, in1=tmp, op=ALU.subtract)
        # nmask = 1 - mask;  t2 += nmask so unowned rows stay > 0
        nc.vector.tensor_scalar(
            out=nmask, in0=tmask_sb[:], scalar1=-1.0, scalar2=1.0,
            op0=ALU.mult, op1=ALU.add,
        )
        nc.vector.tensor_tensor(out=t2, in0=t2, in1=nmask, op=ALU.add)
        # root = sqrt(t2) = exp(0.5*ln(t2))
        nc.scalar.activation(out=root, in_=t2, func=ACTF.Ln)
        nc.scalar.activation(out=root, in_=root, func=ACTF.Exp, scale=0.5)
        nc.vector.tensor_scalar_mul(amc, a, COS_M)
        nc.vector.scalar_tensor_tensor(
            out=margin, in0=root, scalar=-SIN_M, in1=amc, op0=ALU.mult, op1=ALU.add
        )
        # masked AG contributions: margin_m, egl_m = mask*exp(gl), etop_m = mask*exp(margin)
        nc.scalar.activation(out=tmp, in_=glog[:], func=ACTF.Exp)
        nc.vector.tensor_tensor(
            out=pack[:, 2 * BT : 3 * BT], in0=tmp, in1=tmask_sb[:], op=ALU.mult
        )
        nc.scalar.activation(out=tmp, in_=margin, func=ACTF.Exp)
        nc.vector.tensor_tensor(
            out=pack[:, 3 * BT : 4 * BT], in0=tmp, in1=tmask_sb[:], op=ALU.mult
        )
        nc.vector.tensor_tensor(
            out=pack[:, BT : 2 * BT], in0=margin, in1=tmask_sb[:], op=ALU.mult
        )

        # ---- main loop: matmul -> exp (+row-sum accumulate) ----
        for g, group in enumerate(GROUPS):
            wtile = wchunks[g]
            gw = sum(group)
            for bt in range(BT):
                ps = pp.tile([128, 2048], f32, tag="psum", name=f"ps_{g}_{bt}")
                off = 0
                for n in group:
                    nc.tensor.matmul(
                        out=ps[:, off : off + n],
                        lhsT=fT[:, bt * 128 : (bt + 1) * 128],
                        rhs=wtile[:, off : off + n],
                        start=True, stop=True,
                    )
                    off += n
                ex = scratch.tile([128, 2048], f32, tag="exp", name=f"ex_{g}_{bt}")
                col = bt * NG + g
                nc.scalar.activation(
                    out=ex[:, :gw], in_=ps[:, :gw], func=ACTF.Exp,
                    accum_out=acc[:, col : col + 1],
                )

        # ---- pack row-sum partials and AllGather across the 8 cores ----
        for bt in range(BT):
            nc.vector.reduce_sum(
                out=pack[:, bt : bt + 1],
                in_=acc[:, bt * NG : (bt + 1) * NG],
                axis=AX.X,
            )

        cc_in = dp.tile([128, NPACK], f32, tag="cc_in")
        cc_out = dp.tile([NCORES, 128, NPACK], f32, tag="cc_out", addr_space="Shared")
        nc.sync.dma_start(cc_in[:], pack[:])
        nc.gpsimd.collective_compute(
            "AllGather", ALU.bypass,
            replica_groups=[list(range(NCORES))],
            ins=[cc_in[:].opt()], outs=[cc_out[:].opt()],
        )
        # red8[p, m*NPACK + j] = cc_out[m, p, j]
        nc.sync.dma_start(red8[:], cc_out[:].rearrange("m p j -> p m j"))

        # ---- post-collective tail (replicated on every core) ----
        red = epi[:, 7 * BT : 7 * BT + NPACK]  # summed over cores
        nc.vector.reduce_sum(
            out=red,
            in_=red8[:].rearrange("p (m j) -> p j m", m=NCORES),
            axis=AX.X,
        )
        rs = red[:, 0:BT]
        marg = red[:, BT : 2 * BT]
        egl = red[:, 2 * BT : 3 * BT]
        etop = red[:, 3 * BT : 4 * BT]
        down = lane(0)  # reuse
        nc.vector.tensor_tensor(out=down, in0=rs, in1=egl, op=ALU.subtract)
        nc.vector.tensor_tensor(out=down, in0=down, in1=etop, op=ALU.add)
        ld = lane(1)
        nc.scalar.activation(out=ld, in_=down, func=ACTF.Ln)
        val = lane(2)
        nc.vector.tensor_tensor(out=val, in0=marg, in1=ld, op=ALU.subtract)
        colsum = lane(3)[:, 0:1]
        nc.vector.reduce_sum(out=colsum, in_=val, axis=AX.X)
        ps = pp.tile([128, 2048], f32, tag="psum", name="ps_final")
        nc.tensor.matmul(
            out=ps[:1, 0:1], lhsT=colsum, rhs=ones[:], start=True, stop=True
        )
        nc.scalar.mul(res_sb[:], ps[:1, 0:1], -1.0 / B)
        nc.sync.dma_start(out[:, :], res_sb[:])


_CACHED_NC = None


def build():
    global _CACHED_NC
    if _CACHED_NC is not None:
        return _CACHED_NC
    nc = bacc.Bacc(
        "TRN2", target_bir_lowering=False, debug=False, num_devices=NCORES
    )
    feat = nc.dram_tensor("features", [B, F], f32, kind="ExternalInput")
    w = nc.dram_tensor("w", [F, CS], f32, kind="ExternalInput")
    wt = nc.dram_tensor("wt", [CS, F], f32, kind="ExternalInput")
    tidx = nc.dram_tensor("tidx", [128, BT], i32, kind="ExternalInput")
    tmask = nc.dram_tensor("tmask", [128, BT], f32, kind="ExternalInput")
    out = nc.dram_tensor("out", [1, 1], f32, kind="ExternalOutput")
    with tile.TileContext(nc) as tc:
        _body(tc, feat, w, wt, tidx, tmask, out)
    nc.compile()
    _CACHED_NC = nc
    return nc


def make_in_maps(features, w, target):
    features = np.ascontiguousarray(np.asarray(features, dtype=np.float32))
    w = np.asarray(w, dtype=np.float32)
    tgt = np.asarray(target).astype(np.int64).ravel()
    in_maps = []
    for m in range(NCORES):
        base = m * CS
        local = (tgt >= base) & (tgt < base + CS)
        tid = np.where(local, tgt - base, 0).astype(np.int32)
        msk = local.astype(np.float32)
        wshard = np.ascontiguousarray(w[:, base : base + CS])
        in_maps.append(
            {
                "features": features,
                "w": wshard,
                "wt": np.ascontiguousarray(wshard.T),
                # [128, BT] b-major: [p, t] -> row t*128+p
                "tidx": np.ascontiguousarray(tid.reshape(BT, 128).T),
                "tmask": np.ascontiguousarray(msk.reshape(BT, 128).T),
            }
        )
    return in_maps


def run(features, w, target, **kwargs):
    nc = build()
    in_maps = make_in_maps(features, w, target)
    return run_bass_kernel_spmd(nc, in_maps, core_ids=list(range(NCORES)), **kwargs)


def kernel(features, w, target):
    res = run(features, w, target)
    val = np.float32(res.results[0]["out"][0, 0])
    return np.array(val, dtype=np.float32)
